# revision 11
# baseline (speedup 1.0000x reference)
"""Trainium2 Bass kernel for Tacotron-style location attention.

Computes, for B=128, T=2000, D_Q=1024, D_MEM=512, D_ATT=128:
    pq      = hidden @ Wq.T                         # (B, A)
    E[b,t]  = v . tanh(pq[b] + pm[b,t])             # (B, T)
    W       = softmax(where(mask, -inf, E), axis=1) # (B, T)
    ctx     = einsum('bt,btd->bd', W, memory)       # (B, D_MEM)
returns (ctx, W).

Sharding: data-parallel over batch across 8 NeuronCores (16 rows/core);
Wq and v replicated. Per-core design (memory-bound, ~82MB/core HBM traffic):
  - T tiled into 16 chunks of 125; pm/memory DMA'd per batch row as big
    (125p, 16c, feat) slabs (contiguous 512B/2KB runs, one dma_start each).
  - energies stay in the natural (t-partition, a-free) layout: GpSimd adds a
    partition-broadcast copy of pq[b], ScalarE applies tanh, and VectorE
    tensor_tensor_reduce contracts with a partition-broadcast v into energy
    columns e_t[tc, c, b].  No PE matmuls here (fp32 matmul is 2-pass
    LOW_HIGH on PE and per-instruction overhead made a PE-reduction variant
    3x slower than the roofline).
  - 16 small PE transposes flip e_t into row layout (16p, T) for a fully
    vectorized masked softmax along the free axis.
  - context: row-form accumulating matmuls with 1-column weight loads:
    ctx[b] (1,512 PSUM) += W_col(b,c).T @ mem_chunk(b,c) over 16 chunks.
"""

import numpy as np

import concourse.bass as bass
from concourse import bacc
import concourse.mybir as mybir
import concourse.tile as tile
from concourse.bass_utils import run_bass_kernel_spmd
from concourse.masks import make_identity

N_CORES = 8
B, T, DQ, DM, A = 128, 2000, 1024, 512, 128
BL = B // N_CORES          # 16 batch rows per core
TC = 125                   # t-chunk size (T = 16 * 125, no tail)
NCH = T // TC              # 16 chunks
NG = 4                     # chunk groups of 4 -> (125, 4*128) slabs
NEG_BIG = -1.0e38          # masked fill; exp underflows to exactly 0.0

F32 = mybir.dt.float32
U8 = mybir.dt.uint8


def build_nc() -> bass.Bass:
    nc = bacc.Bacc("TRN2", target_bir_lowering=False, debug=False, num_devices=N_CORES)

    hid = nc.dram_tensor("hid", (BL, DQ), F32, kind="ExternalInput")
    mem = nc.dram_tensor("mem", (BL, T, DM), F32, kind="ExternalInput")
    pm = nc.dram_tensor("pm", (BL, T, A), F32, kind="ExternalInput")
    msk = nc.dram_tensor("msk", (BL, T), U8, kind="ExternalInput")
    wq = nc.dram_tensor("wq", (A, DQ), F32, kind="ExternalInput")
    vv = nc.dram_tensor("v", (1, A), F32, kind="ExternalInput")
    ctx_out = nc.dram_tensor("ctx_out", (BL, DM), F32, kind="ExternalOutput")
    w_out = nc.dram_tensor("w_out", (BL, T), F32, kind="ExternalOutput")
    pq_dram = nc.dram_tensor("pq_scratch", (BL, A), F32, kind="Internal")

    with tile.TileContext(nc) as tc:
        with (
            tc.tile_pool(name="singles", bufs=1) as singles,
            tc.tile_pool(name="pmp", bufs=3) as pmp,
            tc.tile_pool(name="memp", bufs=3) as memp,
            tc.tile_pool(name="pqbp", bufs=2) as pqbp,
            tc.tile_pool(name="ttp", bufs=3) as ttp,
            tc.tile_pool(name="thp", bufs=3) as thp,
            tc.tile_pool(name="scr", bufs=2) as scr,
            tc.tile_pool(name="crow", bufs=2) as crow,
            tc.tile_pool(name="ps_tr", bufs=2, space="PSUM") as ps_tr,
            tc.tile_pool(name="ps_ctx", bufs=2, space="PSUM") as ps_ctx,
        ):
            # ---- constants / small loads ----
            idt = singles.tile([128, 128], F32)
            make_identity(nc, idt)
            wq_sb = singles.tile([A, DQ], F32)
            nc.sync.dma_start(out=wq_sb, in_=wq[:, :])
            hid_sb = singles.tile([BL, DQ], F32)
            nc.sync.dma_start(out=hid_sb, in_=hid[:, :])
            msk_sb = singles.tile([BL, T], U8)
            nc.sync.dma_start(out=msk_sb, in_=msk[:, :])
            # v broadcast to TC partitions, replicated NG times along free
            v_bc = singles.tile([TC, NG, A], F32)
            v_ap = vv[:, :]
            nc.gpsimd.dma_start(
                out=v_bc,
                in_=bass.AP(tensor=v_ap.tensor, offset=v_ap.offset,
                            ap=[[0, TC], [0, NG], [1, A]]),
            )

            # ---- pqT[b,a] = sum_d hid[b,d] * Wq[a,d] (PSUM accum, 8 k-chunks)
            wqT = singles.tile([128, DQ], F32)   # (d_local, a) per k-chunk
            hidT = singles.tile([128, 8, BL], F32)  # (d_local, k, b)
            for k in range(8):
                t1 = ps_tr.tile([128, 512], F32, tag="tr")
                nc.tensor.transpose(t1[:, :128], wq_sb[:, bass.ts(k, 128)], idt)
                nc.vector.tensor_copy(wqT[:, bass.ts(k, 128)], t1[:, :128])
                t2 = ps_tr.tile([128, 512], F32, tag="tr")
                nc.tensor.transpose(
                    t2[:, :BL], hid_sb[:, bass.ts(k, 128)], idt[:BL, :BL]
                )
                nc.vector.tensor_copy(hidT[:, k, :], t2[:, :BL])
            pq_ps = ps_tr.tile([128, 512], F32, tag="tr")
            for k in range(8):
                nc.tensor.matmul(
                    pq_ps[:BL, :A],
                    hidT[:, k, :],
                    wqT[:, bass.ts(k, 128)],
                    start=(k == 0),
                    stop=(k == 7),
                    skip_group_check=True,
                )
            pqT = singles.tile([BL, A], F32)
            nc.vector.tensor_copy(pqT, pq_ps[:BL, :A])
            nc.sync.dma_start(out=pq_dram[:, :], in_=pqT)

            # masked fill values: 0.0 where keep, -1e38 where masked
            maskneg = singles.tile([BL, T], F32)
            nc.vector.tensor_scalar_mul(maskneg, msk_sb, NEG_BIG)

            # ---- phase 1: energy columns e_t[tc, c, b]
            e_t = singles.tile([TC, NCH, BL], F32)
            for b in range(BL):
                pm_b = pmp.tile([TC, NCH, A], F32)
                nc.sync.dma_start(
                    out=pm_b, in_=pm[b, :, :].rearrange("(c p) a -> p c a", p=TC)
                )
                # pq[b] broadcast to (TC, NG, A), replicated from DRAM scratch
                pqb = pqbp.tile([TC, NG, A], F32)
                row = pq_dram[b : b + 1, :]
                nc.gpsimd.dma_start(
                    out=pqb,
                    in_=bass.AP(tensor=row.tensor, offset=row.offset,
                                ap=[[0, TC], [0, NG], [1, A]]),
                )
                for g in range(NG):
                    tt = ttp.tile([TC, NG, A], F32)
                    nc.gpsimd.tensor_add(
                        tt, pm_b[:, bass.ts(g, NG), :], pqb
                    )
                    th = thp.tile([TC, NG, A], F32)
                    nc.scalar.activation(
                        th, tt, mybir.ActivationFunctionType.Tanh
                    )
                    s2 = scr.tile([TC, NG, A], F32)
                    nc.vector.tensor_mul(s2, th, v_bc)
                    for j in range(NG):
                        cc = g * NG + j
                        nc.vector.reduce_sum(
                            e_t[:, cc, b : b + 1],
                            s2[:, j, :],
                            axis=mybir.AxisListType.X,
                        )

            # ---- flip e_t to rows: e_sb (16p, 2000)
            e_sb = singles.tile([BL, T], F32)
            for c in range(NCH):
                te = ps_tr.tile([128, 512], F32, tag="tr")
                nc.tensor.transpose(te[:BL, :TC], e_t[:, c, :], idt[:TC, :TC])
                nc.vector.tensor_copy(e_sb[:, bass.ts(c, TC)], te[:BL, :TC])

            # ---- masked softmax over T (free axis), rows = batch
            nc.vector.tensor_add(e_sb, e_sb, maskneg)
            negmax = singles.tile([BL, 1], F32)
            nc.vector.reduce_max(
                negmax, e_sb, axis=mybir.AxisListType.X, negate=True
            )
            w_sb = singles.tile([BL, T], F32)
            rowsum = singles.tile([BL, 1], F32)
            nc.scalar.activation(
                w_sb,
                e_sb,
                mybir.ActivationFunctionType.Exp,
                bias=negmax,
                accum_out=rowsum,
            )
            rinv = singles.tile([BL, 1], F32)
            nc.vector.reciprocal(rinv, rowsum)
            nc.vector.tensor_scalar_mul(w_sb, w_sb, rinv)
            nc.sync.dma_start(out=w_out[:, :], in_=w_sb)

            # ---- transpose W chunks into columns: wt_all[:, c*16+b]
            wt_all = singles.tile([TC, NCH * BL], F32)
            for c in range(NCH):
                tw = ps_tr.tile([128, 512], F32, tag="tr")
                nc.tensor.transpose(
                    tw[:TC, :BL], w_sb[:, bass.ts(c, TC)], idt[:BL, :BL]
                )
                nc.vector.tensor_copy(wt_all[:, bass.ts(c, BL)], tw[:TC, :BL])

            # ---- phase 2: ctx[b] = sum_c W_chunk(b,c) . mem_chunk(b,c)
            for b in range(BL):
                mem_b = memp.tile([TC, NCH, DM], F32)
                nc.sync.dma_start(
                    out=mem_b, in_=mem[b, :, :].rearrange("(c p) d -> p c d", p=TC)
                )
                ctx_acc = ps_ctx.tile([1, DM], F32, tag="ctx")
                for c in range(NCH):
                    nc.tensor.matmul(
                        ctx_acc[:, :],
                        wt_all[:, c * BL + b : c * BL + b + 1],
                        mem_b[:, c, :],
                        start=(c == 0),
                        stop=(c == NCH - 1),
                        skip_group_check=True,
                    )
                ctx_row = crow.tile([1, DM], F32)
                nc.vector.tensor_copy(ctx_row, ctx_acc)
                nc.sync.dma_start(out=ctx_out[b : b + 1, :], in_=ctx_row)

    nc.finalize()
    return nc


_NC_CACHE: list = []


def _get_nc() -> bass.Bass:
    if not _NC_CACHE:
        _NC_CACHE.append(build_nc())
    return _NC_CACHE[0]


def make_in_maps(inputs: dict) -> list:
    ahs = np.ascontiguousarray(np.asarray(inputs["attention_hidden_state"], np.float32))
    memory = np.asarray(inputs["memory"], np.float32)
    pm = np.asarray(inputs["processed_memory"], np.float32)
    mask = np.asarray(inputs["mask"]).astype(np.uint8)
    wq = np.ascontiguousarray(np.asarray(inputs["Wq"], np.float32))
    v = np.ascontiguousarray(np.asarray(inputs["v"], np.float32))
    in_maps = []
    for i in range(N_CORES):
        sl = slice(i * BL, (i + 1) * BL)
        in_maps.append(
            {
                "hid": np.ascontiguousarray(ahs[sl]),
                "mem": np.ascontiguousarray(memory[sl]),
                "pm": np.ascontiguousarray(pm[sl]),
                "msk": np.ascontiguousarray(mask[sl]),
                "wq": wq,
                "v": v,
            }
        )
    return in_maps


def run(inputs: dict, trace: bool = False):
    """Run on 8 cores; returns ((ctx, weights), BassKernelResults)."""
    nc = _get_nc()
    res = run_bass_kernel_spmd(
        nc, make_in_maps(inputs), list(range(N_CORES)), trace=trace
    )
    ctx = np.concatenate([r["ctx_out"] for r in res.results], axis=0)
    w = np.concatenate([r["w_out"] for r in res.results], axis=0)
    return (ctx, w), res


def kernel(**inputs):
    (ctx, w), _ = run(inputs, trace=False)
    return ctx, w


# revision 12
# speedup vs baseline: 1.0646x; 1.0646x over previous
"""Trainium2 Bass kernel for Tacotron-style location attention.

Computes, for B=128, T=2000, D_Q=1024, D_MEM=512, D_ATT=128:
    pq      = hidden @ Wq.T                         # (B, A)
    E[b,t]  = v . tanh(pq[b] + pm[b,t])             # (B, T)
    W       = softmax(where(mask, -inf, E), axis=1) # (B, T)
    ctx     = einsum('bt,btd->bd', W, memory)       # (B, D_MEM)
returns (ctx, W).

Sharding: data-parallel over batch across 8 NeuronCores (16 rows/core);
Wq and v replicated. Per-core design (memory-bound, ~82MB/core HBM traffic):
  - T tiled into 16 chunks of 125; pm/memory DMA'd per batch row as big
    (125p, 16c, feat) slabs (contiguous 512B/2KB runs, one dma_start each).
  - energies stay in the natural (t-partition, a-free) layout: GpSimd adds a
    partition-broadcast copy of pq[b], ScalarE applies tanh, and VectorE
    tensor_tensor_reduce contracts with a partition-broadcast v into energy
    columns e_t[tc, c, b].  No PE matmuls here (fp32 matmul is 2-pass
    LOW_HIGH on PE and per-instruction overhead made a PE-reduction variant
    3x slower than the roofline).
  - 16 small PE transposes flip e_t into row layout (16p, T) for a fully
    vectorized masked softmax along the free axis.
  - context: row-form accumulating matmuls with 1-column weight loads:
    ctx[b] (1,512 PSUM) += W_col(b,c).T @ mem_chunk(b,c) over 16 chunks.
"""

import numpy as np

import concourse.bass as bass
from concourse import bacc
import concourse.mybir as mybir
import concourse.tile as tile
from concourse.bass_utils import run_bass_kernel_spmd
from concourse.masks import make_identity

N_CORES = 8
B, T, DQ, DM, A = 128, 2000, 1024, 512, 128
BL = B // N_CORES          # 16 batch rows per core
TC = 125                   # t-chunk size (T = 16 * 125, no tail)
NCH = T // TC              # 16 chunks
NG = 4                     # chunk groups of 4 -> (125, 4*128) slabs
NEG_BIG = -1.0e38          # masked fill; exp underflows to exactly 0.0

F32 = mybir.dt.float32
U8 = mybir.dt.uint8


def build_nc() -> bass.Bass:
    nc = bacc.Bacc("TRN2", target_bir_lowering=False, debug=False, num_devices=N_CORES)

    hid = nc.dram_tensor("hid", (BL, DQ), F32, kind="ExternalInput")
    mem = nc.dram_tensor("mem", (BL, T, DM), F32, kind="ExternalInput")
    pm = nc.dram_tensor("pm", (BL, T, A), F32, kind="ExternalInput")
    msk = nc.dram_tensor("msk", (BL, T), U8, kind="ExternalInput")
    wq = nc.dram_tensor("wq", (A, DQ), F32, kind="ExternalInput")
    vv = nc.dram_tensor("v", (1, A), F32, kind="ExternalInput")
    ctx_out = nc.dram_tensor("ctx_out", (BL, DM), F32, kind="ExternalOutput")
    w_out = nc.dram_tensor("w_out", (BL, T), F32, kind="ExternalOutput")
    pq_dram = nc.dram_tensor("pq_scratch", (BL, A), F32, kind="Internal")

    with tile.TileContext(nc) as tc:
        with (
            tc.tile_pool(name="singles", bufs=1) as singles,
            tc.tile_pool(name="pmp", bufs=3) as pmp,
            tc.tile_pool(name="memp", bufs=3) as memp,
            tc.tile_pool(name="pqbp", bufs=2) as pqbp,
            tc.tile_pool(name="ttp", bufs=3) as ttp,
            tc.tile_pool(name="thp", bufs=3) as thp,
            tc.tile_pool(name="scr", bufs=2) as scr,
            tc.tile_pool(name="crow", bufs=2) as crow,
            tc.tile_pool(name="ps_tr", bufs=2, space="PSUM") as ps_tr,
            tc.tile_pool(name="ps_ctx", bufs=2, space="PSUM") as ps_ctx,
        ):
            # ---- constants / small loads ----
            idt = singles.tile([128, 128], F32)
            make_identity(nc, idt)
            wq_sb = singles.tile([A, DQ], F32)
            nc.sync.dma_start(out=wq_sb, in_=wq[:, :])
            hid_sb = singles.tile([BL, DQ], F32)
            nc.sync.dma_start(out=hid_sb, in_=hid[:, :])
            msk_sb = singles.tile([BL, T], U8)
            nc.sync.dma_start(out=msk_sb, in_=msk[:, :])
            # v broadcast to TC partitions, replicated NG times along free
            v_bc = singles.tile([TC, NG, A], F32)
            v_ap = vv[:, :]
            nc.gpsimd.dma_start(
                out=v_bc,
                in_=bass.AP(tensor=v_ap.tensor, offset=v_ap.offset,
                            ap=[[0, TC], [0, NG], [1, A]]),
            )

            # ---- pqT[b,a] = sum_d hid[b,d] * Wq[a,d] (PSUM accum, 8 k-chunks)
            wqT = singles.tile([128, DQ], F32)   # (d_local, a) per k-chunk
            hidT = singles.tile([128, 8, BL], F32)  # (d_local, k, b)
            for k in range(8):
                t1 = ps_tr.tile([128, 512], F32, tag="tr")
                nc.tensor.transpose(t1[:, :128], wq_sb[:, bass.ts(k, 128)], idt)
                nc.vector.tensor_copy(wqT[:, bass.ts(k, 128)], t1[:, :128])
                t2 = ps_tr.tile([128, 512], F32, tag="tr")
                nc.tensor.transpose(
                    t2[:, :BL], hid_sb[:, bass.ts(k, 128)], idt[:BL, :BL]
                )
                nc.vector.tensor_copy(hidT[:, k, :], t2[:, :BL])
            pq_ps = ps_tr.tile([128, 512], F32, tag="tr")
            for k in range(8):
                nc.tensor.matmul(
                    pq_ps[:BL, :A],
                    hidT[:, k, :],
                    wqT[:, bass.ts(k, 128)],
                    start=(k == 0),
                    stop=(k == 7),
                    skip_group_check=True,
                )
            pqT = singles.tile([BL, A], F32)
            nc.vector.tensor_copy(pqT, pq_ps[:BL, :A])
            nc.sync.dma_start(out=pq_dram[:, :], in_=pqT)

            # masked fill values: 0.0 where keep, -1e38 where masked
            maskneg = singles.tile([BL, T], F32)
            nc.vector.tensor_scalar_mul(maskneg, msk_sb, NEG_BIG)

            # ---- phase 1: energy columns e_t[tc, c, b]
            e_t = singles.tile([TC, NCH, BL], F32)
            for b in range(BL):
                pm_b = pmp.tile([TC, NCH, A], F32)
                nc.gpsimd.dma_start(
                    out=pm_b, in_=pm[b, :, :].rearrange("(c p) a -> p c a", p=TC)
                )
                # pq[b] broadcast to (TC, NG, A), replicated from DRAM scratch
                pqb = pqbp.tile([TC, NG, A], F32)
                row = pq_dram[b : b + 1, :]
                nc.gpsimd.dma_start(
                    out=pqb,
                    in_=bass.AP(tensor=row.tensor, offset=row.offset,
                                ap=[[0, TC], [0, NG], [1, A]]),
                )
                for g in range(NG):
                    tt = ttp.tile([TC, NG, A], F32)
                    nc.gpsimd.tensor_add(
                        tt, pm_b[:, bass.ts(g, NG), :], pqb
                    )
                    th = thp.tile([TC, NG, A], F32)
                    nc.scalar.activation(
                        th, tt, mybir.ActivationFunctionType.Tanh
                    )
                    s2 = scr.tile([TC, NG, A], F32)
                    nc.vector.tensor_mul(s2, th, v_bc)
                    for j in range(NG):
                        cc = g * NG + j
                        nc.vector.reduce_sum(
                            e_t[:, cc, b : b + 1],
                            s2[:, j, :],
                            axis=mybir.AxisListType.X,
                        )

            # ---- flip e_t to rows: e_sb (16p, 2000)
            e_sb = singles.tile([BL, T], F32)
            for c in range(NCH):
                te = ps_tr.tile([128, 512], F32, tag="tr")
                nc.tensor.transpose(te[:BL, :TC], e_t[:, c, :], idt[:TC, :TC])
                nc.vector.tensor_copy(e_sb[:, bass.ts(c, TC)], te[:BL, :TC])

            # ---- masked softmax over T (free axis), rows = batch
            nc.vector.tensor_add(e_sb, e_sb, maskneg)
            negmax = singles.tile([BL, 1], F32)
            nc.vector.reduce_max(
                negmax, e_sb, axis=mybir.AxisListType.X, negate=True
            )
            w_sb = singles.tile([BL, T], F32)
            rowsum = singles.tile([BL, 1], F32)
            nc.scalar.activation(
                w_sb,
                e_sb,
                mybir.ActivationFunctionType.Exp,
                bias=negmax,
                accum_out=rowsum,
            )
            rinv = singles.tile([BL, 1], F32)
            nc.vector.reciprocal(rinv, rowsum)
            nc.vector.tensor_scalar_mul(w_sb, w_sb, rinv)
            nc.sync.dma_start(out=w_out[:, :], in_=w_sb)

            # ---- transpose W chunks into columns: wt_all[:, c*16+b]
            wt_all = singles.tile([TC, NCH * BL], F32)
            for c in range(NCH):
                tw = ps_tr.tile([128, 512], F32, tag="tr")
                nc.tensor.transpose(
                    tw[:TC, :BL], w_sb[:, bass.ts(c, TC)], idt[:BL, :BL]
                )
                nc.vector.tensor_copy(wt_all[:, bass.ts(c, BL)], tw[:TC, :BL])

            # ---- phase 2: ctx[b] = sum_c W_chunk(b,c) . mem_chunk(b,c)
            for b in range(BL):
                mem_b = memp.tile([TC, NCH, DM], F32)
                mem_view = mem[b, :, :].rearrange("(c p) d -> p c d", p=TC)
                half = NCH // 2
                nc.sync.dma_start(
                    out=mem_b[:, :half, :], in_=mem_view[:, :half, :]
                )
                nc.scalar.dma_start(
                    out=mem_b[:, half:, :], in_=mem_view[:, half:, :]
                )
                ctx_acc = ps_ctx.tile([1, DM], F32, tag="ctx")
                for c in range(NCH):
                    nc.tensor.matmul(
                        ctx_acc[:, :],
                        wt_all[:, c * BL + b : c * BL + b + 1],
                        mem_b[:, c, :],
                        start=(c == 0),
                        stop=(c == NCH - 1),
                        skip_group_check=True,
                    )
                ctx_row = crow.tile([1, DM], F32)
                nc.vector.tensor_copy(ctx_row, ctx_acc)
                nc.sync.dma_start(out=ctx_out[b : b + 1, :], in_=ctx_row)

    nc.finalize()
    return nc


_NC_CACHE: list = []


def _get_nc() -> bass.Bass:
    if not _NC_CACHE:
        _NC_CACHE.append(build_nc())
    return _NC_CACHE[0]


def make_in_maps(inputs: dict) -> list:
    ahs = np.ascontiguousarray(np.asarray(inputs["attention_hidden_state"], np.float32))
    memory = np.asarray(inputs["memory"], np.float32)
    pm = np.asarray(inputs["processed_memory"], np.float32)
    mask = np.asarray(inputs["mask"]).astype(np.uint8)
    wq = np.ascontiguousarray(np.asarray(inputs["Wq"], np.float32))
    v = np.ascontiguousarray(np.asarray(inputs["v"], np.float32))
    in_maps = []
    for i in range(N_CORES):
        sl = slice(i * BL, (i + 1) * BL)
        in_maps.append(
            {
                "hid": np.ascontiguousarray(ahs[sl]),
                "mem": np.ascontiguousarray(memory[sl]),
                "pm": np.ascontiguousarray(pm[sl]),
                "msk": np.ascontiguousarray(mask[sl]),
                "wq": wq,
                "v": v,
            }
        )
    return in_maps


def run(inputs: dict, trace: bool = False):
    """Run on 8 cores; returns ((ctx, weights), BassKernelResults)."""
    nc = _get_nc()
    res = run_bass_kernel_spmd(
        nc, make_in_maps(inputs), list(range(N_CORES)), trace=trace
    )
    ctx = np.concatenate([r["ctx_out"] for r in res.results], axis=0)
    w = np.concatenate([r["w_out"] for r in res.results], axis=0)
    return (ctx, w), res


def kernel(**inputs):
    (ctx, w), _ = run(inputs, trace=False)
    return ctx, w
